# revision 8
# baseline (speedup 1.0000x reference)
"""Trainium2 Bass kernel for nn_AffineCouplingBlock_6287832121753.

Self-contained: `kernel(**inputs)` takes the FULL inputs, shards the batch
dim over 8 NeuronCores (pure data parallel), runs a Bass/Tile kernel via
run_bass_kernel_spmd, and gathers the full (z, log_det) output.

Per-core device program (per batch, per coupling):
  - index/bilinear-weight pipeline on [128, 32] f32 tiles (sample-block major)
  - gather indices folded to the DMA-gather's [16, 256] slot layout via 8
    selection matmuls on the tensor engine
  - one dma_gather (4096 idxs x 512B) from a host-prebuilt "quad" table that
    stores all 4 bilinear corners per row
  - corner weighting via per-partition-scalar fused multiply-accumulate chains
  - comb tiles ([64ch lf + z_keep] x 128 samples) built by DMA-xbar transpose
  - 5-layer MLP in bf16 on the tensor engine (silu on ScalarE, biases folded
    into the activation bias; cond term folded into the L0 bias on host)
  - reversed final matmuls put s,t back into sample-partition layout;
    exp(1.5*tanh(s)) computed as a DVE polynomial (no exp table switch)
"""
import numpy as np

B, T = 64, 4096
NCORES = 8
NB = B // NCORES
P = 128
NBLK = T // P
TQ_ROWS = 4104
ELEM = 256

_CACHE = {}


def _build_program(nc, n_batches=NB, mlp_chunk=2048, repeat=1):
    import concourse.mybir as mybir
    from concourse import library_config
    from concourse.tile import TileContext

    F32 = mybir.dt.float32
    BF16 = mybir.dt.bfloat16
    I16 = mybir.dt.int16
    AF = mybir.ActivationFunctionType
    OP = mybir.AluOpType

    NBb = n_batches
    n_chunks = T // mlp_chunk

    xa_d = nc.dram_tensor("xa", [NBb, P, NBLK], F32, kind="ExternalInput")
    xb_d = nc.dram_tensor("xb", [NBb, P, NBLK], F32, kind="ExternalInput")
    tq_d = nc.dram_tensor("tq", [NBb, TQ_ROWS, ELEM], BF16, kind="ExternalInput")
    sel_d = nc.dram_tensor("sel", [P, 8 * P], F32, kind="ExternalInput")
    w0s_d = nc.dram_tensor("w0s", [P, 4 * P], BF16, kind="ExternalInput")
    w1_d = nc.dram_tensor("w1", [P, 12 * P], BF16, kind="ExternalInput")
    w2_d = nc.dram_tensor("w2", [P, 4], BF16, kind="ExternalInput")
    b0f_d = nc.dram_tensor("b0f", [NBb, P, 4], F32, kind="ExternalInput")
    b1t_d = nc.dram_tensor("b1t", [P, 12], F32, kind="ExternalInput")
    b2t_d = nc.dram_tensor("b2t", [P, 4], F32, kind="ExternalInput")
    zout_d = nc.dram_tensor("zout", [NBb, 2, P, NBLK], F32, kind="ExternalOutput")
    ldout_d = nc.dram_tensor("ldout", [NBb, P, NBLK], F32, kind="ExternalOutput")

    with TileContext(nc) as tc:
        nc.gpsimd.load_library(library_config.mlp)
        with (
            tc.tile_pool(name="const", bufs=1) as cpool,
            tc.tile_pool(name="io", bufs=3) as iopool,
            tc.tile_pool(name="gather", bufs=3) as gpool,
            tc.tile_pool(name="stage", bufs=3) as spool,
            tc.tile_pool(name="mlp", bufs=5) as hpool,
            tc.tile_pool(name="pipe", bufs=2) as ppool,
            tc.tile_pool(name="psum", bufs=2, space="PSUM") as psum_pool,
        ):
            sel_t = cpool.tile([P, 8 * P], F32, tag="sel")
            nc.sync.dma_start(out=sel_t[:], in_=sel_d[:])
            w0s_t = cpool.tile([P, 4 * P], BF16, tag="w0s")
            nc.sync.dma_start(out=w0s_t[:], in_=w0s_d[:])
            w1_t = cpool.tile([P, 12 * P], BF16, tag="w1")
            nc.sync.dma_start(out=w1_t[:], in_=w1_d[:])
            w2_t = cpool.tile([P, 4], BF16, tag="w2")
            nc.sync.dma_start(out=w2_t[:], in_=w2_d[:])
            b1t_t = cpool.tile([P, 12], F32, tag="b1t")
            nc.sync.dma_start(out=b1t_t[:], in_=b1t_d[:])
            b2t_t = cpool.tile([P, 4], F32, tag="b2t")
            nc.sync.dma_start(out=b2t_t[:], in_=b2t_d[:])
            # staging tiles reused round-robin; memset once so the xbar
            # transpose never reads uninitialized rows 65..127
            stage_ab = []
            for si in range(3):
                st = cpool.tile([P, T], BF16, tag=f"stage{si}", name=f"stage{si}")
                nc.vector.memset(st[:], 0.0)
                stage_ab.append(st)

            for _rep in range(repeat):
              for b in range(NBb):
                xa_t = iopool.tile([P, NBLK], F32, tag="xa")
                nc.sync.dma_start(out=xa_t[:], in_=xa_d[b])
                xb_t = iopool.tile([P, NBLK], F32, tag="xb")
                nc.sync.dma_start(out=xb_t[:], in_=xb_d[b])
                b0f_t = iopool.tile([P, 4], F32, tag="b0f")
                nc.sync.dma_start(out=b0f_t[:], in_=b0f_d[b])

                za_new = None
                zb_new = None
                ld_t = None

                for cpl in range(2):
                    if cpl == 0:
                        cx, cy, zkeep = xa_t, xb_t, xb_t
                    else:
                        cx, cy, zkeep = za_new, xb_t, za_new

                    # ---------- index/weight pipeline ([128, 32] f32) ----------
                    def axis_pipeline(c_t, name):
                        shs = ppool.tile([P, NBLK], F32, tag=f"{name}shs",
                                         name=f"{name}shs_{b}")
                        nc.vector.tensor_scalar(
                            out=shs[:], in0=c_t[:], scalar1=63.0, scalar2=1.0,
                            op0=OP.mult, op1=OP.add)
                        nc.vector.tensor_scalar(
                            out=shs[:], in0=shs[:], scalar1=0.0, scalar2=65.0,
                            op0=OP.max, op1=OP.min)
                        # floor via int cast + correction (works for either
                        # trunc or round-to-nearest cast semantics; shs >= 0)
                        i32 = ppool.tile([P, NBLK], mybir.dt.int32,
                                         tag=f"{name}i32", name=f"{name}i32_{b}")
                        nc.vector.tensor_copy(out=i32[:], in_=shs[:])
                        icf = ppool.tile([P, NBLK], F32, tag=f"{name}icf",
                                         name=f"{name}icf_{b}")
                        nc.vector.tensor_copy(out=icf[:], in_=i32[:])
                        gt = ppool.tile([P, NBLK], F32, tag=f"{name}gt",
                                        name=f"{name}gt_{b}")
                        nc.vector.tensor_tensor(
                            out=gt[:], in0=icf[:], in1=shs[:], op=OP.is_gt)
                        i0 = ppool.tile([P, NBLK], F32, tag=f"{name}i0",
                                        name=f"{name}i0_{b}")
                        nc.vector.tensor_tensor(
                            out=i0[:], in0=icf[:], in1=gt[:], op=OP.subtract)
                        fr = ppool.tile([P, NBLK], F32, tag=f"{name}fr",
                                        name=f"{name}fr_{b}")
                        nc.vector.tensor_tensor(
                            out=fr[:], in0=shs[:], in1=i0[:], op=OP.subtract)
                        m0 = ppool.tile([P, NBLK], F32, tag=f"{name}m0",
                                        name=f"{name}m0_{b}")
                        nc.vector.tensor_scalar(
                            out=m0[:], in0=i0[:], scalar1=1.0, scalar2=None,
                            op0=OP.is_ge)
                        m1 = ppool.tile([P, NBLK], F32, tag=f"{name}m1",
                                        name=f"{name}m1_{b}")
                        nc.vector.tensor_scalar(
                            out=m1[:], in0=i0[:], scalar1=64.0, scalar2=None,
                            op0=OP.is_le)
                        a0 = ppool.tile([P, NBLK], F32, tag=f"{name}a0",
                                        name=f"{name}a0_{b}")
                        nc.vector.tensor_scalar(
                            out=a0[:], in0=fr[:], scalar1=-1.0, scalar2=1.0,
                            op0=OP.mult, op1=OP.add)
                        nc.vector.tensor_tensor(
                            out=a0[:], in0=a0[:], in1=m0[:], op=OP.mult)
                        nc.vector.tensor_tensor(
                            out=a0[:], in0=a0[:], in1=m1[:], op=OP.mult)
                        a1 = ppool.tile([P, NBLK], F32, tag=f"{name}a1",
                                        name=f"{name}a1_{b}")
                        nc.vector.tensor_scalar(
                            out=a1[:], in0=i0[:], scalar1=63.0, scalar2=None,
                            op0=OP.is_le)
                        nc.vector.tensor_tensor(
                            out=a1[:], in0=a1[:], in1=fr[:], op=OP.mult)
                        ic = ppool.tile([P, NBLK], F32, tag=f"{name}ic",
                                        name=f"{name}ic_{b}")
                        nc.vector.tensor_scalar(
                            out=ic[:], in0=i0[:], scalar1=1.0, scalar2=64.0,
                            op0=OP.max, op1=OP.min)
                        # low-clip (-1 -> 0): valid i0+1 corner's value sits in
                        # the FIRST gathered texel slot; move a1's weight there
                        mlow = ppool.tile([P, NBLK], F32, tag=f"{name}mlow",
                                          name=f"{name}mlow_{b}")
                        nc.vector.tensor_scalar(
                            out=mlow[:], in0=i0[:], scalar1=0.5, scalar2=None,
                            op0=OP.is_le)
                        nc.vector.tensor_tensor(
                            out=mlow[:], in0=mlow[:], in1=a1[:], op=OP.mult)
                        g0 = ppool.tile([P, NBLK], F32, tag=f"{name}g0",
                                        name=f"{name}g0_{b}")
                        nc.vector.tensor_tensor(
                            out=g0[:], in0=a0[:], in1=mlow[:], op=OP.add)
                        g1 = ppool.tile([P, NBLK], F32, tag=f"{name}g1",
                                        name=f"{name}g1_{b}")
                        nc.vector.tensor_tensor(
                            out=g1[:], in0=a1[:], in1=mlow[:], op=OP.subtract)
                        return g0, g1, ic

                    ax0, ax1, xc = axis_pipeline(cx, f"x{cpl}")
                    by0, by1, yc = axis_pipeline(cy, f"y{cpl}")

                    r_t = ppool.tile([P, NBLK], F32, tag="r", name=f"r_{b}_{cpl}")
                    nc.vector.tensor_scalar(
                        out=r_t[:], in0=yc[:], scalar1=64.0, scalar2=-65.0,
                        op0=OP.mult, op1=OP.add)
                    nc.vector.tensor_tensor(
                        out=r_t[:], in0=r_t[:], in1=xc[:], op=OP.add)

                    e00 = ppool.tile([P, NBLK], F32, tag="e00", name=f"e00_{b}_{cpl}")
                    nc.vector.tensor_tensor(out=e00[:], in0=by0[:], in1=ax0[:], op=OP.mult)
                    e10 = ppool.tile([P, NBLK], F32, tag="e10", name=f"e10_{b}_{cpl}")
                    nc.vector.tensor_tensor(out=e10[:], in0=by1[:], in1=ax0[:], op=OP.mult)
                    e01 = ppool.tile([P, NBLK], F32, tag="e01", name=f"e01_{b}_{cpl}")
                    nc.vector.tensor_tensor(out=e01[:], in0=by0[:], in1=ax1[:], op=OP.mult)
                    e11 = ppool.tile([P, NBLK], F32, tag="e11", name=f"e11_{b}_{cpl}")
                    nc.vector.tensor_tensor(out=e11[:], in0=by1[:], in1=ax1[:], op=OP.mult)

                    # ---------- gather index build: 8 selection matmuls ----------
                    idx_ps = psum_pool.tile([P, 8 * NBLK], F32, tag="mlp",
                                            name=f"idxps_{b}_{cpl}")
                    for u in range(8):
                        nc.tensor.matmul(
                            out=idx_ps[:, u * NBLK:(u + 1) * NBLK],
                            lhsT=sel_t[:, u * P:(u + 1) * P],
                            rhs=r_t[:],
                            start=True, stop=True)
                    idx16 = gpool.tile([P, 256], I16, tag="idx", name=f"idx_{b}_{cpl}")
                    src_v = idx_ps[:].rearrange("p (u j) -> p u j", u=8)
                    dst_v = idx16[:].rearrange("p (j u) -> p u j", u=8)
                    nc.vector.tensor_copy(out=dst_v, in_=src_v)

                    # ---------- gather ----------
                    g_t = gpool.tile([P, NBLK, ELEM], BF16, tag="g",
                                     name=f"g_{b}_{cpl}")
                    nc.gpsimd.dma_gather(
                        out_ap=g_t[:],
                        in_ap=tq_d[b],
                        idxs_ap=idx16[:],
                        num_idxs=T,
                        num_idxs_reg=T,
                        elem_size=ELEM,
                        single_packet=False)

                    # ---------- weighting + staging ----------
                    stage_t = stage_ab[(2 * b + cpl) % 3]
                    import concourse.mybir as _mb
                    tmpw = [ppool.tile([P, 64], BF16, tag=f"tw{i}",
                                       name=f"tw{i}_{b}_{cpl}") for i in range(3)]
                    for n in range(NBLK):
                        nc.vector.tensor_scalar(
                            out=tmpw[0][:], in0=g_t[:, n, 0:64],
                            scalar1=e00[:, n:n + 1], scalar2=None, op0=OP.mult)
                        nc.vector.scalar_tensor_tensor(
                            out=tmpw[1][:], in0=g_t[:, n, 64:128],
                            scalar=e10[:, n:n + 1], in1=tmpw[0][:],
                            op0=OP.mult, op1=OP.add)
                        nc.vector.scalar_tensor_tensor(
                            out=tmpw[2][:], in0=g_t[:, n, 128:192],
                            scalar=e01[:, n:n + 1], in1=tmpw[1][:],
                            op0=OP.mult, op1=OP.add)
                        nc.vector.scalar_tensor_tensor(
                            out=stage_t[:, n * P:n * P + 64], in0=g_t[:, n, 192:256],
                            scalar=e11[:, n:n + 1], in1=tmpw[2][:],
                            op0=OP.mult, op1=OP.add)
                    nc.vector.tensor_copy(out=stage_t[:, 64:T:P], in_=zkeep[:])

                    # ---------- transpose staging -> comb tiles ----------
                    tmega = spool.tile([P, T], BF16, tag="tmega",
                                       name=f"tmega_{b}_{cpl}")
                    for n in range(NBLK):
                        nc.sync.dma_start_transpose(
                            out=tmega[:, n * P:(n + 1) * P],
                            in_=stage_t[:, n * P:(n + 1) * P])

                    # ---------- MLP (2 nets) ----------
                    h4 = {}
                    for net in range(2):
                        gnet = 2 * cpl + net
                        ha = hpool.tile([P, T], BF16, tag="h",
                                        name=f"h0_{b}_{cpl}_{net}")
                        for ch in range(n_chunks):
                            ps = psum_pool.tile([P, mlp_chunk], F32, tag="mlp",
                                                name=f"ps0_{b}_{cpl}_{net}_{ch}")
                            for k in range(mlp_chunk // 512):
                                lo = ch * mlp_chunk + k * 512
                                nc.tensor.matmul(
                                    out=ps[:, k * 512:(k + 1) * 512],
                                    lhsT=w0s_t[0:65, gnet * P:(gnet + 1) * P],
                                    rhs=tmega[0:65, lo:lo + 512],
                                    start=True, stop=True)
                            nc.scalar.activation(
                                out=ha[:, ch * mlp_chunk:(ch + 1) * mlp_chunk],
                                in_=ps[:], func=AF.Silu,
                                bias=b0f_t[:, gnet:gnet + 1], scale=1.0)
                        for layer in range(3):
                            hb = hpool.tile([P, T], BF16, tag="h",
                                            name=f"h{layer + 1}_{b}_{cpl}_{net}")
                            w_ap = w1_t[:, (gnet * 3 + layer) * P:
                                        (gnet * 3 + layer + 1) * P]
                            for ch in range(n_chunks):
                                ps = psum_pool.tile(
                                    [P, mlp_chunk], F32, tag="mlp",
                                    name=f"ps{layer + 1}_{b}_{cpl}_{net}_{ch}")
                                for k in range(mlp_chunk // 512):
                                    lo = ch * mlp_chunk + k * 512
                                    nc.tensor.matmul(
                                        out=ps[:, k * 512:(k + 1) * 512],
                                        lhsT=w_ap,
                                        rhs=ha[:, lo:lo + 512],
                                        start=True, stop=True)
                                nc.scalar.activation(
                                    out=hb[:, ch * mlp_chunk:(ch + 1) * mlp_chunk],
                                    in_=ps[:], func=AF.Silu,
                                    bias=b1t_t[:, gnet * 3 + layer:
                                               gnet * 3 + layer + 1],
                                    scale=1.0)
                            ha = hb
                        h4[net] = ha

                    # ---------- reversed finals -> [128 samples, 2] ----------
                    st_ps = psum_pool.tile([P, 2 * NBLK], F32, tag="mlp",
                                           name=f"stps_{b}_{cpl}")
                    for n in range(NBLK):
                        for net in range(2):
                            nc.tensor.matmul(
                                out=st_ps[:, 2 * n + net:2 * n + net + 1],
                                lhsT=h4[net][:, n * P:(n + 1) * P],
                                rhs=w2_t[:, 2 * cpl + net:2 * cpl + net + 1],
                                start=True, stop=True)

                    s_raw_t = ppool.tile([P, NBLK], F32, tag="sraw",
                                         name=f"sraw_{b}_{cpl}")
                    nc.vector.tensor_scalar(
                        out=s_raw_t[:], in0=st_ps[:, 0:2 * NBLK:2],
                        scalar1=b2t_t[:, 2 * cpl:2 * cpl + 1], scalar2=None,
                        op0=OP.add)
                    t_val_t = ppool.tile([P, NBLK], F32, tag="tval",
                                         name=f"tval_{b}_{cpl}")
                    nc.vector.tensor_scalar(
                        out=t_val_t[:], in0=st_ps[:, 1:2 * NBLK:2],
                        scalar1=b2t_t[:, 2 * cpl + 1:2 * cpl + 2], scalar2=None,
                        op0=OP.add)

                    # ---------- tail ----------
                    th = ppool.tile([P, NBLK], F32, tag="tanh", name=f"th_{b}_{cpl}")
                    nc.scalar.activation(out=th[:], in_=s_raw_t[:], func=AF.Tanh,
                                         scale=1.0)
                    if cpl == 0:
                        ld_t = ppool.tile([P, NBLK], F32, tag="ld", name=f"ld_{b}")
                        nc.vector.tensor_scalar(
                            out=ld_t[:], in0=th[:], scalar1=1.5, scalar2=None,
                            op0=OP.mult)
                    else:
                        nc.vector.scalar_tensor_tensor(
                            out=ld_t[:], in0=th[:], scalar=1.5, in1=ld_t[:],
                            op0=OP.mult, op1=OP.add)
                    # E = exp(1.5*th): deg-4 poly of z=1.5*th/8, then ^8
                    zq = ppool.tile([P, NBLK], F32, tag="zq", name=f"zq_{b}_{cpl}")
                    nc.vector.tensor_scalar(
                        out=zq[:], in0=th[:], scalar1=0.1875, scalar2=None,
                        op0=OP.mult)
                    q_t = ppool.tile([P, NBLK], F32, tag="qt", name=f"qt_{b}_{cpl}")
                    nc.vector.tensor_tensor(out=q_t[:], in0=zq[:], in1=zq[:], op=OP.mult)
                    a_t = ppool.tile([P, NBLK], F32, tag="at", name=f"at_{b}_{cpl}")
                    nc.vector.tensor_scalar(
                        out=a_t[:], in0=zq[:], scalar1=1.0 / 6.0, scalar2=0.5,
                        op0=OP.mult, op1=OP.add)
                    b_t = ppool.tile([P, NBLK], F32, tag="bt", name=f"bt_{b}_{cpl}")
                    nc.vector.tensor_tensor(out=b_t[:], in0=q_t[:], in1=a_t[:], op=OP.mult)
                    c_t = ppool.tile([P, NBLK], F32, tag="ct", name=f"ct_{b}_{cpl}")
                    nc.vector.scalar_tensor_tensor(
                        out=c_t[:], in0=zq[:], scalar=1.0, in1=b_t[:],
                        op0=OP.add, op1=OP.add)
                    p4 = ppool.tile([P, NBLK], F32, tag="p4", name=f"p4_{b}_{cpl}")
                    nc.vector.tensor_tensor(out=p4[:], in0=q_t[:], in1=q_t[:], op=OP.mult)
                    e_t = ppool.tile([P, NBLK], F32, tag="et", name=f"et_{b}_{cpl}")
                    nc.vector.scalar_tensor_tensor(
                        out=e_t[:], in0=p4[:], scalar=1.0 / 24.0, in1=c_t[:],
                        op0=OP.mult, op1=OP.add)
                    for _ in range(3):
                        nc.vector.tensor_tensor(out=e_t[:], in0=e_t[:], in1=e_t[:],
                                                op=OP.mult)

                    zk = xa_t if cpl == 0 else xb_t
                    zn = ppool.tile([P, NBLK], F32, tag=f"zn{cpl}",
                                    name=f"zn{cpl}_{b}")
                    nc.vector.tensor_tensor(out=zn[:], in0=zk[:], in1=e_t[:], op=OP.mult)
                    nc.vector.tensor_tensor(out=zn[:], in0=zn[:], in1=t_val_t[:],
                                            op=OP.add)
                    if cpl == 0:
                        za_new = zn
                    else:
                        zb_new = zn

                nc.sync.dma_start(out=zout_d[b, 0], in_=za_new[:])
                nc.sync.dma_start(out=zout_d[b, 1], in_=zb_new[:])
                nc.sync.dma_start(out=ldout_d[b], in_=ld_t[:])

    return nc


def _build():
    if "nc" in _CACHE:
        return _CACHE["nc"]
    import concourse.bacc as bacc
    nc = bacc.Bacc("TRN2", target_bir_lowering=False, debug=False)
    _build_program(nc, n_batches=NB)
    nc.compile()
    _CACHE["nc"] = nc
    return nc


# ---------------- host-side preprocessing ----------------

def _mk_quad_table(w_b):
    import ml_dtypes
    whwc = np.transpose(w_b, (1, 2, 0))
    wp = np.pad(whwc, ((0, 1), (0, 1), (0, 0)))
    quad = np.concatenate(
        [wp[0:64, 0:64], wp[1:65, 0:64], wp[0:64, 1:65], wp[1:65, 1:65]],
        axis=-1).reshape(4096, 256)
    out = np.zeros((TQ_ROWS, ELEM), dtype=ml_dtypes.bfloat16)
    out[:4096] = quad.astype(ml_dtypes.bfloat16)
    return out


def _mk_sel():
    sel = np.zeros((P, 8 * P), dtype=np.float32)
    for u in range(8):
        for Pi in range(P):
            sel[16 * u + (Pi % 16), u * P + Pi] = 1.0
    return sel


def _host_inputs_for_core(x_sh, w_sh, cond_sh, W0, b0, W1, b1, W2, b2):
    import ml_dtypes
    nb = x_sh.shape[0]
    xa = np.ascontiguousarray(
        x_sh[:, :, 0].reshape(nb, NBLK, P).transpose(0, 2, 1)).astype(np.float32)
    xb = np.ascontiguousarray(
        x_sh[:, :, 1].reshape(nb, NBLK, P).transpose(0, 2, 1)).astype(np.float32)
    tq = np.stack([_mk_quad_table(w_sh[i]) for i in range(nb)])

    w0s = np.zeros((P, 4 * P), dtype=ml_dtypes.bfloat16)
    for net in range(4):
        blk = np.zeros((P, P), dtype=np.float32)
        blk[0:64, :] = W0[net][:, 1:65].T
        blk[64, :] = W0[net][:, 0]
        w0s[:, net * P:(net + 1) * P] = blk.astype(ml_dtypes.bfloat16)
    w1 = np.zeros((P, 12 * P), dtype=ml_dtypes.bfloat16)
    for net in range(4):
        for layer in range(3):
            w1[:, (net * 3 + layer) * P:(net * 3 + layer + 1) * P] = \
                W1[net, layer].T.astype(ml_dtypes.bfloat16)
    w2 = np.zeros((P, 4), dtype=ml_dtypes.bfloat16)
    for net in range(4):
        w2[:, net] = W2[net, 0].astype(ml_dtypes.bfloat16)
    b0f = np.zeros((nb, P, 4), dtype=np.float32)
    for net in range(4):
        b0f[:, :, net] = cond_sh @ W0[net][:, 65:81].T + b0[net]
    b1t = np.zeros((P, 12), dtype=np.float32)
    for net in range(4):
        for layer in range(3):
            b1t[:, net * 3 + layer] = b1[net, layer]
    b2t = np.zeros((P, 4), dtype=np.float32)
    for net in range(4):
        b2t[:, net] = b2[net, 0]
    return dict(xa=xa, xb=xb, tq=tq, sel=_mk_sel(), w0s=w0s, w1=w1, w2=w2,
                b0f=b0f, b1t=b1t, b2t=b2t)


def _postprocess(results):
    zs, lds = [], []
    for r in results:
        zout = np.asarray(r["zout"], dtype=np.float32)
        ldout = np.asarray(r["ldout"], dtype=np.float32)
        nb = zout.shape[0]
        zt = zout.transpose(0, 1, 3, 2).reshape(nb, 2, T)
        zs.append(np.stack([zt[:, 0], zt[:, 1]], axis=-1))
        lds.append(ldout.reshape(nb, -1).sum(axis=1))
    z = np.concatenate(zs, axis=0).astype(np.float32)
    log_det = np.concatenate(lds, axis=0).astype(np.float32)
    return z, log_det


def kernel(x, w, cond, W0, b0, W1, b1, W2, b2):
    from concourse.bass_utils import run_bass_kernel_spmd

    x = np.asarray(x, dtype=np.float32)
    w = np.asarray(w, dtype=np.float32)
    cond = np.asarray(cond, dtype=np.float32)
    W0 = np.asarray(W0, dtype=np.float32)
    b0 = np.asarray(b0, dtype=np.float32)
    W1 = np.asarray(W1, dtype=np.float32)
    b1 = np.asarray(b1, dtype=np.float32)
    W2 = np.asarray(W2, dtype=np.float32)
    b2 = np.asarray(b2, dtype=np.float32)

    nc = _build()
    in_maps = []
    for c in range(NCORES):
        sl = slice(c * NB, (c + 1) * NB)
        in_maps.append(_host_inputs_for_core(
            x[sl], w[sl], cond[sl], W0, b0, W1, b1, W2, b2))

    res = run_bass_kernel_spmd(nc, in_maps, core_ids=list(range(NCORES)))
    return _postprocess(res.results)


# revision 16
# speedup vs baseline: 96.5787x; 96.5787x over previous
"""Trainium2 Bass kernel for nn_AffineCouplingBlock_6287832121753.

Self-contained: `kernel(**inputs)` takes the FULL inputs, shards the batch
dim over 8 NeuronCores (pure data parallel), runs a Bass/Tile kernel via
run_bass_kernel_spmd, and gathers the full (z, log_det) output.

Per-core device program, per (batch, coupling) unit:
  S1: index/bilinear-weight pipeline on [128, 32] f32 tiles; gather indices
      folded to the DMA-gather slot layout via 8 selection matmuls; one
      dma_gather (4096 idxs x 512B) from a host-prebuilt 4-corner quad table
  S2: corner weighting via per-partition-scalar fused MAC chains into a bf16
      staging tile ([64ch lf + z_keep row] per 128-sample block); comb tiles
      built by DMA-xbar transposes
  S3: 5-layer MLP in bf16 on the tensor engine (silu on ScalarE with biases
      in the activation bias; cond term folded into the L0 bias on host);
      reversed final matmuls return s,t in sample-partition layout
  S4: tail: tanh on ScalarE, exp(1.5*tanh) as a DVE polynomial (all ACT work
      stays in the silu table set), z update, log-det accumulation

All coupling-0 units (independent across batches) are emitted before
coupling-1 units, and stages are software-pipelined via scheduler priorities
so different units' stages overlap across engines.
"""
import numpy as np

B, T = 64, 4096
NCORES = 8
NB = B // NCORES
P = 128
NBLK = T // P
TQ_ROWS = 4104
ELEM = 256

_CACHE = {}


def _build_program(nc, n_batches=NB, repeat=1, stages=(1, 2, 3, 4), tx_blocks=32):
    import concourse.mybir as mybir
    from concourse import library_config
    from concourse.tile import TileContext

    F32 = mybir.dt.float32
    BF16 = mybir.dt.bfloat16
    I16 = mybir.dt.int16
    AF = mybir.ActivationFunctionType
    OP = mybir.AluOpType

    NBb = n_batches
    CHUNKS = ((0, 2048), (2048, 2048))
    SLOT = 1 << 21
    STAGE_RANK = {1: 3, 2: 2, 3: 1, 4: 0}

    xa_d = nc.dram_tensor("xa", [NBb, P, NBLK], F32, kind="ExternalInput")
    xb_d = nc.dram_tensor("xb", [NBb, P, NBLK], F32, kind="ExternalInput")
    tq_d = nc.dram_tensor("tq", [NBb, TQ_ROWS, ELEM], BF16, kind="ExternalInput")
    sel_d = nc.dram_tensor("sel", [P, 8 * P], F32, kind="ExternalInput")
    w0s_d = nc.dram_tensor("w0s", [P, 4 * P], BF16, kind="ExternalInput")
    w1_d = nc.dram_tensor("w1", [P, 12 * P], BF16, kind="ExternalInput")
    w2_d = nc.dram_tensor("w2", [P, 4], BF16, kind="ExternalInput")
    b0f_d = nc.dram_tensor("b0f", [NBb, P, 4], F32, kind="ExternalInput")
    b1t_d = nc.dram_tensor("b1t", [P, 12], F32, kind="ExternalInput")
    b2t_d = nc.dram_tensor("b2t", [P, 4], F32, kind="ExternalInput")
    zout_d = nc.dram_tensor("zout", [NBb, 2, P, NBLK], F32, kind="ExternalOutput")
    ldout_d = nc.dram_tensor("ldout", [NBb, P, NBLK], F32, kind="ExternalOutput")

    with TileContext(nc) as tc:
        nc.gpsimd.load_library(library_config.mlp)
        with (
            tc.tile_pool(name="const", bufs=1) as cpool,
            tc.tile_pool(name="io", bufs=2) as iopool,
            tc.tile_pool(name="gather", bufs=3) as gpool,
            tc.tile_pool(name="stage", bufs=3) as spool,
            tc.tile_pool(name="mlp", bufs=5) as hpool,
            tc.tile_pool(name="pipe", bufs=3) as ppool,
            tc.tile_pool(name="psum", bufs=2, space="PSUM") as psum_pool,
        ):
            def prio(uidx, stage):
                tc.cur_priority = ((uidx + stage - 1) * SLOT
                                   + STAGE_RANK[stage] * (SLOT // 8))

            tc.cur_priority = 0
            sel_t = cpool.tile([P, 8 * P], F32, tag="sel")
            nc.sync.dma_start(out=sel_t[:], in_=sel_d[:])
            w0s_t = cpool.tile([P, 4 * P], BF16, tag="w0s")
            nc.sync.dma_start(out=w0s_t[:], in_=w0s_d[:])
            w1_t = cpool.tile([P, 12 * P], BF16, tag="w1")
            nc.sync.dma_start(out=w1_t[:], in_=w1_d[:])
            w2_t = cpool.tile([P, 4], BF16, tag="w2")
            nc.sync.dma_start(out=w2_t[:], in_=w2_d[:])
            b1t_t = cpool.tile([P, 12], F32, tag="b1t")
            nc.sync.dma_start(out=b1t_t[:], in_=b1t_d[:])
            b2t_t = cpool.tile([P, 4], F32, tag="b2t")
            nc.sync.dma_start(out=b2t_t[:], in_=b2t_d[:])
            # staging tiles reused round-robin; memset once so the xbar
            # transpose never reads uninitialized rows 65..127
            stage_ab = []
            for si in range(3):
                st = cpool.tile([P, T], BF16, tag=f"stage{si}", name=f"stage{si}")
                nc.vector.memset(st[:], 0.0)
                stage_ab.append(st)

            state = {}

            def axis_pipeline(c_t, name, b):
                shs = ppool.tile([P, NBLK], F32, tag=f"{name}shs",
                                 name=f"{name}shs_{b}")
                nc.vector.tensor_scalar(
                    out=shs[:], in0=c_t[:], scalar1=63.0, scalar2=1.0,
                    op0=OP.mult, op1=OP.add)
                nc.vector.tensor_scalar(
                    out=shs[:], in0=shs[:], scalar1=0.0, scalar2=65.0,
                    op0=OP.max, op1=OP.min)
                # floor via int cast + correction (robust to cast rounding)
                i32 = ppool.tile([P, NBLK], mybir.dt.int32,
                                 tag=f"{name}i32", name=f"{name}i32_{b}")
                nc.vector.tensor_copy(out=i32[:], in_=shs[:])
                icf = ppool.tile([P, NBLK], F32, tag=f"{name}icf",
                                 name=f"{name}icf_{b}")
                nc.vector.tensor_copy(out=icf[:], in_=i32[:])
                gt = ppool.tile([P, NBLK], F32, tag=f"{name}gt",
                                name=f"{name}gt_{b}")
                nc.vector.tensor_tensor(
                    out=gt[:], in0=icf[:], in1=shs[:], op=OP.is_gt)
                i0 = ppool.tile([P, NBLK], F32, tag=f"{name}i0",
                                name=f"{name}i0_{b}")
                nc.vector.tensor_tensor(
                    out=i0[:], in0=icf[:], in1=gt[:], op=OP.subtract)
                fr = ppool.tile([P, NBLK], F32, tag=f"{name}fr",
                                name=f"{name}fr_{b}")
                nc.vector.tensor_tensor(
                    out=fr[:], in0=shs[:], in1=i0[:], op=OP.subtract)
                m0 = ppool.tile([P, NBLK], F32, tag=f"{name}m0",
                                name=f"{name}m0_{b}")
                nc.vector.tensor_scalar(
                    out=m0[:], in0=i0[:], scalar1=1.0, scalar2=None,
                    op0=OP.is_ge)
                m1 = ppool.tile([P, NBLK], F32, tag=f"{name}m1",
                                name=f"{name}m1_{b}")
                nc.vector.tensor_scalar(
                    out=m1[:], in0=i0[:], scalar1=64.0, scalar2=None,
                    op0=OP.is_le)
                a0 = ppool.tile([P, NBLK], F32, tag=f"{name}a0",
                                name=f"{name}a0_{b}")
                nc.vector.tensor_scalar(
                    out=a0[:], in0=fr[:], scalar1=-1.0, scalar2=1.0,
                    op0=OP.mult, op1=OP.add)
                nc.vector.tensor_tensor(
                    out=a0[:], in0=a0[:], in1=m0[:], op=OP.mult)
                nc.vector.tensor_tensor(
                    out=a0[:], in0=a0[:], in1=m1[:], op=OP.mult)
                a1 = ppool.tile([P, NBLK], F32, tag=f"{name}a1",
                                name=f"{name}a1_{b}")
                nc.vector.tensor_scalar(
                    out=a1[:], in0=i0[:], scalar1=63.0, scalar2=None,
                    op0=OP.is_le)
                nc.vector.tensor_tensor(
                    out=a1[:], in0=a1[:], in1=fr[:], op=OP.mult)
                ic = ppool.tile([P, NBLK], F32, tag=f"{name}ic",
                                name=f"{name}ic_{b}")
                nc.vector.tensor_scalar(
                    out=ic[:], in0=i0[:], scalar1=1.0, scalar2=64.0,
                    op0=OP.max, op1=OP.min)
                # low-clip (-1 -> 0): valid i0+1 corner's value sits in the
                # FIRST gathered texel slot; move a1's weight there
                mlow = ppool.tile([P, NBLK], F32, tag=f"{name}mlow",
                                  name=f"{name}mlow_{b}")
                nc.vector.tensor_scalar(
                    out=mlow[:], in0=i0[:], scalar1=0.5, scalar2=None,
                    op0=OP.is_le)
                nc.vector.tensor_tensor(
                    out=mlow[:], in0=mlow[:], in1=a1[:], op=OP.mult)
                g0 = ppool.tile([P, NBLK], F32, tag=f"{name}g0",
                                name=f"{name}g0_{b}")
                nc.vector.tensor_tensor(
                    out=g0[:], in0=a0[:], in1=mlow[:], op=OP.add)
                g1 = ppool.tile([P, NBLK], F32, tag=f"{name}g1",
                                name=f"{name}g1_{b}")
                nc.vector.tensor_tensor(
                    out=g1[:], in0=a1[:], in1=mlow[:], op=OP.subtract)
                return g0, g1, ic

            def emit_unit(b, cpl, uidx):
                # ---------------- S1: pipeline + idx build + gather ----------
                prio(uidx, 1)
                S = stages
                if cpl == 0:
                    xa_t = iopool.tile([P, NBLK], F32, tag=f"xa{b}",
                                       name=f"xa_{b}")
                    nc.sync.dma_start(out=xa_t[:], in_=xa_d[b])
                    xb_t = iopool.tile([P, NBLK], F32, tag=f"xb{b}",
                                       name=f"xb_{b}")
                    nc.sync.dma_start(out=xb_t[:], in_=xb_d[b])
                    b0f_t = iopool.tile([P, 4], F32, tag=f"b0f{b}",
                                        name=f"b0f_{b}")
                    nc.sync.dma_start(out=b0f_t[:], in_=b0f_d[b])
                    state[b] = {"xa": xa_t, "xb": xb_t, "b0f": b0f_t}
                st = state[b]
                xa_t, xb_t, b0f_t = st["xa"], st["xb"], st["b0f"]
                if cpl == 0:
                    cx, cy, zkeep, zk = xa_t, xb_t, xb_t, xa_t
                else:
                    cx, cy, zkeep, zk = st["za"], xb_t, st["za"], xb_t

                if 1 not in S:
                    if cpl == 0:
                        state[b]["za"] = xa_t
                        state[b]["ld"] = xa_t
                    return
                ax0, ax1, xc = axis_pipeline(cx, f"x{cpl}", b)
                by0, by1, yc = axis_pipeline(cy, f"y{cpl}", b)

                r_t = ppool.tile([P, NBLK], F32, tag="r", name=f"r_{b}_{cpl}")
                nc.vector.tensor_scalar(
                    out=r_t[:], in0=yc[:], scalar1=64.0, scalar2=-65.0,
                    op0=OP.mult, op1=OP.add)
                nc.vector.tensor_tensor(
                    out=r_t[:], in0=r_t[:], in1=xc[:], op=OP.add)

                e00 = ppool.tile([P, NBLK], F32, tag="e00", name=f"e00_{b}_{cpl}")
                nc.vector.tensor_tensor(out=e00[:], in0=by0[:], in1=ax0[:], op=OP.mult)
                e10 = ppool.tile([P, NBLK], F32, tag="e10", name=f"e10_{b}_{cpl}")
                nc.vector.tensor_tensor(out=e10[:], in0=by1[:], in1=ax0[:], op=OP.mult)
                e01 = ppool.tile([P, NBLK], F32, tag="e01", name=f"e01_{b}_{cpl}")
                nc.vector.tensor_tensor(out=e01[:], in0=by0[:], in1=ax1[:], op=OP.mult)
                e11 = ppool.tile([P, NBLK], F32, tag="e11", name=f"e11_{b}_{cpl}")
                nc.vector.tensor_tensor(out=e11[:], in0=by1[:], in1=ax1[:], op=OP.mult)

                idx_ps = psum_pool.tile([P, 8 * NBLK], F32, tag="mlp",
                                        name=f"idxps_{b}_{cpl}")
                for u in range(8):
                    nc.tensor.matmul(
                        out=idx_ps[:, u * NBLK:(u + 1) * NBLK],
                        lhsT=sel_t[:, u * P:(u + 1) * P],
                        rhs=r_t[:],
                        start=True, stop=True)
                idx16 = gpool.tile([P, 256], I16, tag="idx", name=f"idx_{b}_{cpl}")
                src_v = idx_ps[:].rearrange("p (u j) -> p u j", u=8)
                dst_v = idx16[:].rearrange("p (j u) -> p u j", u=8)
                nc.vector.tensor_copy(out=dst_v, in_=src_v)

                g_t = gpool.tile([P, NBLK, ELEM], BF16, tag="g",
                                 name=f"g_{b}_{cpl}")
                nc.gpsimd.dma_gather(
                    out_ap=g_t[:],
                    in_ap=tq_d[b],
                    idxs_ap=idx16[:],
                    num_idxs=T,
                    num_idxs_reg=T,
                    elem_size=ELEM,
                    single_packet=False)

                if 2 not in S:
                    if cpl == 0:
                        state[b]["za"] = xa_t
                        state[b]["ld"] = xa_t
                    return
                # ---------------- S2: weighting + staging + transpose --------
                prio(uidx, 2)
                stage_t = stage_ab[uidx % 3]
                tmpw = [ppool.tile([P, 64], BF16, tag=f"tw{i}",
                                   name=f"tw{i}_{b}_{cpl}") for i in range(3)]
                for n in range(NBLK):
                    nc.vector.tensor_scalar(
                        out=tmpw[0][:], in0=g_t[:, n, 0:64],
                        scalar1=e00[:, n:n + 1], scalar2=None, op0=OP.mult)
                    nc.vector.scalar_tensor_tensor(
                        out=tmpw[1][:], in0=g_t[:, n, 64:128],
                        scalar=e10[:, n:n + 1], in1=tmpw[0][:],
                        op0=OP.mult, op1=OP.add)
                    nc.vector.scalar_tensor_tensor(
                        out=tmpw[2][:], in0=g_t[:, n, 128:192],
                        scalar=e01[:, n:n + 1], in1=tmpw[1][:],
                        op0=OP.mult, op1=OP.add)
                    nc.vector.scalar_tensor_tensor(
                        out=stage_t[:, n * P:n * P + 64], in0=g_t[:, n, 192:256],
                        scalar=e11[:, n:n + 1], in1=tmpw[2][:],
                        op0=OP.mult, op1=OP.add)
                nc.vector.tensor_copy(out=stage_t[:, 64:T:P], in_=zkeep[:])

                tmega = spool.tile([P, T], BF16, tag="tmega",
                                   name=f"tmega_{b}_{cpl}")
                for base in range(0, NBLK, tx_blocks):
                    lo = base * P
                    wdt = tx_blocks * P
                    nc.sync.dma_start_transpose(
                        out=tmega[:, lo:lo + wdt].rearrange(
                            "p (n f) -> p n f", f=P),
                        in_=stage_t[:, lo:lo + wdt])

                if 3 not in S:
                    if cpl == 0:
                        state[b]["za"] = xa_t
                        state[b]["ld"] = xa_t
                    return
                # ---------------- S3: MLP (net-interleaved) ------------------
                prio(uidx, 3)
                hcur = {}
                for layer in range(4):
                    for net in range(2):
                        gnet = 2 * cpl + net
                        hb = hpool.tile([P, T], BF16, tag="h",
                                        name=f"h{layer}_{b}_{cpl}_{net}")
                        if layer == 0:
                            w_ap = w0s_t[0:65, gnet * P:(gnet + 1) * P]
                            bias_ap = b0f_t[:, gnet:gnet + 1]
                        else:
                            w_ap = w1_t[:, (gnet * 3 + layer - 1) * P:
                                        (gnet * 3 + layer) * P]
                            bias_ap = b1t_t[:, gnet * 3 + layer - 1:
                                            gnet * 3 + layer]
                        for ch, (clo, csz) in enumerate(CHUNKS):
                            ps = psum_pool.tile(
                                [P, csz], F32, tag="mlp",
                                name=f"ps{layer}_{b}_{cpl}_{net}_{ch}")
                            for k in range(csz // 512):
                                lo = clo + k * 512
                                if layer == 0:
                                    rhs = tmega[0:65, lo:lo + 512]
                                else:
                                    rhs = hcur[net][:, lo:lo + 512]
                                nc.tensor.matmul(
                                    out=ps[:, k * 512:(k + 1) * 512],
                                    lhsT=w_ap, rhs=rhs,
                                    start=True, stop=True)
                            nc.scalar.activation(
                                out=hb[:, clo:clo + csz],
                                in_=ps[:], func=AF.Silu,
                                bias=bias_ap, scale=1.0)
                        hcur[net] = hb
                h4 = hcur

                st_ps = psum_pool.tile([P, 2 * NBLK], F32, tag="mlp",
                                       name=f"stps_{b}_{cpl}")
                for n in range(NBLK):
                    for net in range(2):
                        nc.tensor.matmul(
                            out=st_ps[:, 2 * n + net:2 * n + net + 1],
                            lhsT=h4[net][:, n * P:(n + 1) * P],
                            rhs=w2_t[:, 2 * cpl + net:2 * cpl + net + 1],
                            start=True, stop=True)

                if 4 not in S:
                    if cpl == 0:
                        state[b]["za"] = xa_t
                        state[b]["ld"] = xa_t
                    return
                # ---------------- S4: tail ----------------------------------
                prio(uidx, 4)
                t_val_t = ppool.tile([P, NBLK], F32, tag="tval",
                                     name=f"tval_{b}_{cpl}")
                nc.vector.tensor_scalar(
                    out=t_val_t[:], in0=st_ps[:, 1:2 * NBLK:2],
                    scalar1=b2t_t[:, 2 * cpl + 1:2 * cpl + 2], scalar2=None,
                    op0=OP.add)
                # tanh(s_raw + b2) straight from PSUM with the bias folded in
                th = ppool.tile([P, NBLK], F32, tag="tanh", name=f"th_{b}_{cpl}")
                nc.scalar.activation(out=th[:], in_=st_ps[:, 0:2 * NBLK:2],
                                     func=AF.Tanh,
                                     bias=b2t_t[:, 2 * cpl:2 * cpl + 1],
                                     scale=1.0)
                if cpl == 0:
                    ld_t = iopool.tile([P, NBLK], F32, tag=f"ld{b}",
                                       name=f"ld_{b}")
                    nc.vector.tensor_scalar(
                        out=ld_t[:], in0=th[:], scalar1=1.5, scalar2=None,
                        op0=OP.mult)
                    state[b]["ld"] = ld_t
                else:
                    ld_t = state[b]["ld"]
                    nc.vector.scalar_tensor_tensor(
                        out=ld_t[:], in0=th[:], scalar=1.5, in1=ld_t[:],
                        op0=OP.mult, op1=OP.add)
                # E = exp(1.5*th): deg-4 poly of z = 1.5*th/8, then ^8
                zq = ppool.tile([P, NBLK], F32, tag="zq", name=f"zq_{b}_{cpl}")
                nc.vector.tensor_scalar(
                    out=zq[:], in0=th[:], scalar1=0.1875, scalar2=None,
                    op0=OP.mult)
                q_t = ppool.tile([P, NBLK], F32, tag="qt", name=f"qt_{b}_{cpl}")
                nc.vector.tensor_tensor(out=q_t[:], in0=zq[:], in1=zq[:], op=OP.mult)
                a_t = ppool.tile([P, NBLK], F32, tag="at", name=f"at_{b}_{cpl}")
                nc.vector.tensor_scalar(
                    out=a_t[:], in0=zq[:], scalar1=1.0 / 6.0, scalar2=0.5,
                    op0=OP.mult, op1=OP.add)
                b_t = ppool.tile([P, NBLK], F32, tag="bt", name=f"bt_{b}_{cpl}")
                nc.vector.tensor_tensor(out=b_t[:], in0=q_t[:], in1=a_t[:], op=OP.mult)
                c_t = ppool.tile([P, NBLK], F32, tag="ct", name=f"ct_{b}_{cpl}")
                nc.vector.scalar_tensor_tensor(
                    out=c_t[:], in0=zq[:], scalar=1.0, in1=b_t[:],
                    op0=OP.add, op1=OP.add)
                p4 = ppool.tile([P, NBLK], F32, tag="p4", name=f"p4_{b}_{cpl}")
                nc.vector.tensor_tensor(out=p4[:], in0=q_t[:], in1=q_t[:], op=OP.mult)
                e_t = ppool.tile([P, NBLK], F32, tag="et", name=f"et_{b}_{cpl}")
                nc.vector.scalar_tensor_tensor(
                    out=e_t[:], in0=p4[:], scalar=1.0 / 24.0, in1=c_t[:],
                    op0=OP.mult, op1=OP.add)
                for _ in range(3):
                    nc.vector.tensor_tensor(out=e_t[:], in0=e_t[:], in1=e_t[:],
                                            op=OP.mult)

                zn = iopool.tile([P, NBLK], F32, tag=f"zn{cpl}_{b}",
                                 name=f"zn{cpl}_{b}")
                nc.vector.tensor_tensor(out=zn[:], in0=zk[:], in1=e_t[:], op=OP.mult)
                nc.vector.tensor_tensor(out=zn[:], in0=zn[:], in1=t_val_t[:],
                                        op=OP.add)
                if cpl == 0:
                    state[b]["za"] = zn
                    nc.sync.dma_start(out=zout_d[b, 0], in_=zn[:])
                else:
                    nc.sync.dma_start(out=zout_d[b, 1], in_=zn[:])
                    nc.sync.dma_start(out=ldout_d[b], in_=ld_t[:])

            for rep in range(repeat):
                for cpl in range(2):
                    for b in range(NBb):
                        uidx = rep * 2 * NBb + cpl * NBb + b
                        emit_unit(b, cpl, uidx)

    return nc


def _build():
    if "nc" in _CACHE:
        return _CACHE["nc"]
    import concourse.bacc as bacc
    nc = bacc.Bacc("TRN2", target_bir_lowering=False, debug=False)
    _build_program(nc, n_batches=NB)
    nc.compile()
    _CACHE["nc"] = nc
    return nc


# ---------------- host-side preprocessing ----------------

def _mk_quad_table(w_b):
    import ml_dtypes
    whwc = np.transpose(w_b, (1, 2, 0))
    wp = np.pad(whwc, ((0, 1), (0, 1), (0, 0)))
    quad = np.concatenate(
        [wp[0:64, 0:64], wp[1:65, 0:64], wp[0:64, 1:65], wp[1:65, 1:65]],
        axis=-1).reshape(4096, 256)
    out = np.zeros((TQ_ROWS, ELEM), dtype=ml_dtypes.bfloat16)
    out[:4096] = quad.astype(ml_dtypes.bfloat16)
    return out


def _mk_sel():
    sel = np.zeros((P, 8 * P), dtype=np.float32)
    for u in range(8):
        for Pi in range(P):
            sel[16 * u + (Pi % 16), u * P + Pi] = 1.0
    return sel


def _host_inputs_for_core(x_sh, w_sh, cond_sh, W0, b0, W1, b1, W2, b2):
    import ml_dtypes
    nb = x_sh.shape[0]
    xa = np.ascontiguousarray(
        x_sh[:, :, 0].reshape(nb, NBLK, P).transpose(0, 2, 1)).astype(np.float32)
    xb = np.ascontiguousarray(
        x_sh[:, :, 1].reshape(nb, NBLK, P).transpose(0, 2, 1)).astype(np.float32)
    tq = np.stack([_mk_quad_table(w_sh[i]) for i in range(nb)])

    w0s = np.zeros((P, 4 * P), dtype=ml_dtypes.bfloat16)
    for net in range(4):
        blk = np.zeros((P, P), dtype=np.float32)
        blk[0:64, :] = W0[net][:, 1:65].T
        blk[64, :] = W0[net][:, 0]
        w0s[:, net * P:(net + 1) * P] = blk.astype(ml_dtypes.bfloat16)
    w1 = np.zeros((P, 12 * P), dtype=ml_dtypes.bfloat16)
    for net in range(4):
        for layer in range(3):
            w1[:, (net * 3 + layer) * P:(net * 3 + layer + 1) * P] = \
                W1[net, layer].T.astype(ml_dtypes.bfloat16)
    w2 = np.zeros((P, 4), dtype=ml_dtypes.bfloat16)
    for net in range(4):
        w2[:, net] = W2[net, 0].astype(ml_dtypes.bfloat16)
    b0f = np.zeros((nb, P, 4), dtype=np.float32)
    for net in range(4):
        b0f[:, :, net] = cond_sh @ W0[net][:, 65:81].T + b0[net]
    b1t = np.zeros((P, 12), dtype=np.float32)
    for net in range(4):
        for layer in range(3):
            b1t[:, net * 3 + layer] = b1[net, layer]
    b2t = np.zeros((P, 4), dtype=np.float32)
    for net in range(4):
        b2t[:, net] = b2[net, 0]
    return dict(xa=xa, xb=xb, tq=tq, sel=_mk_sel(), w0s=w0s, w1=w1, w2=w2,
                b0f=b0f, b1t=b1t, b2t=b2t)


def _postprocess(results):
    zs, lds = [], []
    for r in results:
        zout = np.asarray(r["zout"], dtype=np.float32)
        ldout = np.asarray(r["ldout"], dtype=np.float32)
        nb = zout.shape[0]
        zt = zout.transpose(0, 1, 3, 2).reshape(nb, 2, T)
        zs.append(np.stack([zt[:, 0], zt[:, 1]], axis=-1))
        lds.append(ldout.reshape(nb, -1).sum(axis=1))
    z = np.concatenate(zs, axis=0).astype(np.float32)
    log_det = np.concatenate(lds, axis=0).astype(np.float32)
    return z, log_det


def kernel(x, w, cond, W0, b0, W1, b1, W2, b2):
    from concourse.bass_utils import run_bass_kernel_spmd

    x = np.asarray(x, dtype=np.float32)
    w = np.asarray(w, dtype=np.float32)
    cond = np.asarray(cond, dtype=np.float32)
    W0 = np.asarray(W0, dtype=np.float32)
    b0 = np.asarray(b0, dtype=np.float32)
    W1 = np.asarray(W1, dtype=np.float32)
    b1 = np.asarray(b1, dtype=np.float32)
    W2 = np.asarray(W2, dtype=np.float32)
    b2 = np.asarray(b2, dtype=np.float32)

    nc = _build()
    in_maps = []
    for c in range(NCORES):
        sl = slice(c * NB, (c + 1) * NB)
        in_maps.append(_host_inputs_for_core(
            x[sl], w[sl], cond[sl], W0, b0, W1, b1, W2, b2))

    res = run_bass_kernel_spmd(nc, in_maps, core_ids=list(range(NCORES)))
    return _postprocess(res.results)
